# revision 32
# baseline (speedup 1.0000x reference)
"""Trainium2 Bass kernel for nn_NodeModel (GNN scatter-mean + node MLP).

Self-contained: takes FULL inputs as numpy arrays, shards by destination-node
range across 8 NeuronCores, runs a Bass/Tile kernel per core via
run_bass_kernel_spmd, and reassembles the full [500000, 8] output.

Strategy: nodes sharded by destination range (62500/core, no collectives).
The host sorts edges by destination, degree-sorts nodes within each core, and
packs the per-edge message [x[row] | edge_attr] (16 ch, bf16) into per-chunk
slot arrays whose slot count G tracks the local max degree (~33 avg instead of
the global max ~70), laid out partition-major so every stream DMA is
[128 partitions x large-contiguous].  Per-node counts (already computed for
the layout) ship as a tiny side input.

Device per core: chunked DMA -> one reduce_sum per chunk (DVE) over the slot
axis -> mean via max/reciprocal/multiply -> PE transposes of 128-node feature
columns -> PE matmuls for the 2-layer MLP (W1 24x25, W2 25x8, bf16), ACT for
bias+ReLU and PSUM evacuation.  Output is [8, npad] channel-major; the host
transposes and un-permutes the degree sort.
"""
from contextlib import ExitStack

import numpy as np

import concourse.bacc as bacc
import concourse.mybir as mybir
import concourse.tile as tile
from concourse.bass_utils import run_bass_kernel_spmd
from concourse.masks import make_identity

F_X = 8
F_E = 8
NCH = F_X + F_E          # 16 summed message channels
HF = F_X + NCH           # 24 feature channels into the MLP
H = 25
N_CORES = 8
N_NODES = 500_000
N_PER = N_NODES // N_CORES   # 62500
NQ = 4                       # quarters (pipeline granularity)
NPP = 492                    # node columns per core (492*128 = 62976 >= 62500)
L_BUDGET = 16896             # stream elems per partition per chunk
                             # (fp8 in HBM: 16.9KB/partition, 2.16MB per DMA)


def plan_chunks(env, npp, nq, l_budget=L_BUDGET):
    """env: [npp*128] descending max-degree envelope (shared across cores).
    Returns ([(q, col_in_q, C, G, off)], total_W). One chunk = C node columns
    sharing slot count G; per-partition layout [ch][col][slot]."""
    qc = npp // nq
    chunks = []
    off = 0
    for q in range(nq):
        col = 0
        while col < qc:
            g = max(1, int(env[(q * qc + col) * 128]))
            c = max(1, min(qc - col, l_budget // (NCH * g)))
            chunks.append((q, col, c, g, off))
            off += NCH * c * g
            col += c
    return chunks, off


def build_kernel(npp, nq, chunks, W, repeat=1, do_reduce=True, do_mlp=True,
                 st_bufs=2):
    qc = npp // nq
    dt = mybir.dt
    nc = bacc.Bacc("TRN2", target_bir_lowering=False)

    # messages live in HBM as fp8-e4m3 (16 B/edge); the SWDGE cast-DMA
    # expands to bf16 in SBUF so the reduce keeps its 2x 16-bit mode
    streamP = nc.dram_tensor("streamP", [128, W], dt.float8e4,
                             kind="ExternalInput")
    xq = nc.dram_tensor("xq", [128, nq, F_X, qc], dt.float32,
                        kind="ExternalInput")
    cntq = nc.dram_tensor("cntq", [128, nq, qc], dt.float32,
                          kind="ExternalInput")
    w1 = nc.dram_tensor("w1", [HF, H], dt.bfloat16, kind="ExternalInput")
    b1 = nc.dram_tensor("b1", [H, 1], dt.float32, kind="ExternalInput")
    w2 = nc.dram_tensor("w2", [H, F_X], dt.bfloat16, kind="ExternalInput")
    b2 = nc.dram_tensor("b2", [F_X, 1], dt.float32, kind="ExternalInput")
    outP = nc.dram_tensor("outP", [F_X, npp * 128], dt.float32,
                          kind="ExternalOutput")

    st_size = max(L_BUDGET, max(NCH * c * g for (_, _, c, g, _) in chunks))
    relu = mybir.ActivationFunctionType.Relu
    identf = mybir.ActivationFunctionType.Identity

    with tile.TileContext(nc) as tc, ExitStack() as ctx:
        const = ctx.enter_context(tc.tile_pool(name="const", bufs=1))
        persist = ctx.enter_context(tc.tile_pool(name="persist", bufs=1))
        sp = ctx.enter_context(tc.tile_pool(name="stream", bufs=st_bufs))
        msb = ctx.enter_context(tc.tile_pool(name="mlp", bufs=2))
        obp = ctx.enter_context(tc.tile_pool(name="outb", bufs=2))
        psum = ctx.enter_context(tc.tile_pool(name="psum", bufs=2,
                                              space="PSUM"))

        ident = const.tile([128, 128], dt.float32)
        make_identity(nc, ident)
        w1t = const.tile([HF, H], dt.bfloat16)
        nc.sync.dma_start(out=w1t[:], in_=w1[:])
        b1t = const.tile([H, 1], dt.float32)
        nc.sync.dma_start(out=b1t[:], in_=b1[:])
        w2t = const.tile([H, F_X], dt.bfloat16)
        nc.sync.dma_start(out=w2t[:], in_=w2[:])
        b2t = const.tile([F_X, 1], dt.float32)
        nc.sync.dma_start(out=b2t[:], in_=b2[:])

        by_q = {q: [ch for ch in chunks if ch[0] == q] for q in range(nq)}

        for q in [q for _ in range(repeat) for q in range(nq)]:
            feat = persist.tile([128, HF, qc], dt.float32, tag=f"feat{q}")
            accum = persist.tile([128, NCH, qc], dt.float32, tag=f"acc{q}")
            inv = persist.tile([128, qc], dt.float32, tag=f"inv{q}")

            # scalar (ACT) HWDGE ring: keeps these off the SP ring so a
            # queued wait can't stall the stream DMAs behind it
            nc.scalar.dma_start(out=feat[:, 0:F_X, :], in_=xq[:, q])
            nc.scalar.dma_start(out=inv[:], in_=cntq[:, q])
            nc.vector.tensor_scalar_max(out=inv[:], in0=inv[:], scalar1=1.0)
            nc.vector.reciprocal(out=inv[:], in_=inv[:])

            if do_reduce:
                for (_, col, c, g, off) in by_q[q]:
                    stt = sp.tile([128, st_size], dt.bfloat16, tag="st")
                    n = NCH * c * g
                    nc.gpsimd.dma_start(out=stt[:, :n],
                                        in_=streamP[:, off:off + n])
                    nc.vector.reduce_sum(
                        out=accum[:, :, col:col + c],
                        in_=stt[:, :n].rearrange("p (f c g) -> p f c g",
                                                 f=NCH, c=c),
                        axis=mybir.AxisListType.X,
                    )

                for ci in range(NCH):
                    nc.vector.tensor_tensor(
                        out=feat[:, F_X + ci, :], in0=accum[:, ci, :],
                        in1=inv[:], op=mybir.AluOpType.mult,
                    )

            if not do_mlp:  # timing probe only: skip MLP, output stays zero
                continue

            # ---- MLP over this quarter, blocks of up to 4 node columns ----
            ob = None
            ob_base = 0
            for b0 in range(0, qc, 4):
                bc = min(4, qc - b0)
                n = bc * 128
                if (b0 // 4) % 4 == 0:
                    ob = obp.tile([F_X, 2048], dt.float32, tag="ob")
                    ob_base = b0
                ftp = psum.tile([HF, 512], dt.float32, tag="ft")
                for i in range(bc):
                    nc.tensor.transpose(ftp[:, i * 128:(i + 1) * 128],
                                        feat[:, :, b0 + i], ident)
                fts = msb.tile([HF, 512], dt.bfloat16, tag="fts")
                nc.scalar.copy(out=fts[:, :n], in_=ftp[:, :n])
                hp = psum.tile([H, 512], dt.float32, tag="h")
                nc.tensor.matmul(hp[:, :n], w1t[:], fts[:, :n],
                                 start=True, stop=True)
                hs = msb.tile([H, 512], dt.bfloat16, tag="hs")
                nc.scalar.activation(hs[:, :n], hp[:, :n], relu, bias=b1t[:])
                op_ = psum.tile([F_X, 512], dt.float32, tag="o")
                nc.tensor.matmul(op_[:, :n], w2t[:], hs[:, :n],
                                 start=True, stop=True)
                oc = (b0 - ob_base) * 128
                nc.scalar.activation(ob[:, oc:oc + n], op_[:, :n], identf,
                                     bias=b2t[:])
                if (b0 // 4) % 4 == 3 or b0 + bc >= qc:
                    done = (b0 + bc - ob_base) * 128
                    base = (q * qc + ob_base) * 128
                    nc.scalar.dma_start(out=outP[:, base:base + done],
                                        in_=ob[:, :done])

    nc.compile()
    return nc


def _to_bf16(a_f32):
    """f32 -> bf16 (round-to-nearest-even) as uint16 view."""
    u = np.ascontiguousarray(a_f32).view(np.uint32)
    rounded = (u + 0x7FFF + ((u >> 16) & 1)) >> 16
    return rounded.astype(np.uint16)


def _to_fp8(a_f32):
    """f32 -> fp8 e4m3 as uint8 view."""
    import ml_dtypes
    return np.ascontiguousarray(a_f32).astype(ml_dtypes.float8_e4m3).view(
        np.uint8)


def prep_stage1(x, row, col, edge_attr, n_nodes=N_NODES):
    """Layout-independent prep: destination sort + fp8 message table."""
    deg = np.bincount(col, minlength=n_nodes).astype(np.int64)
    order = np.argsort(col.astype(np.int32), kind="stable")
    sc = col.astype(np.int32)[order]
    starts = np.zeros(n_nodes + 1, np.int64)
    starts[1:] = np.cumsum(deg)
    within = np.arange(len(col), dtype=np.int64) - starts[sc]
    x8 = _to_fp8(x.astype(np.float32))
    ea8 = _to_fp8(edge_attr.astype(np.float32))
    msg8 = np.empty((len(col), NCH), np.uint8)
    msg8[:, :F_X] = x8[row[order]]
    msg8[:, F_X:] = ea8[order]
    return dict(deg=deg, sc=sc, within=within, msg8=msg8)


def prep_core_inputs(x, row, col, edge_attr, W1, b1, W2, b2, u,
                     n_nodes=N_NODES, n_cores=N_CORES, npp=NPP, nq=NQ,
                     l_budget=L_BUDGET, stage1=None):
    n_per = n_nodes // n_cores
    npad = npp * 128
    qc = npp // nq
    if stage1 is None:
        stage1 = prep_stage1(x, row, col, edge_attr, n_nodes=n_nodes)
    deg = stage1["deg"]
    sc = stage1["sc"]
    within = stage1["within"]
    msg8 = stage1["msg8"]

    # per-core degree sort; shared descending max-degree envelope
    orders = []
    dsort = np.zeros((n_cores, npad), np.int64)
    for c in range(n_cores):
        d = deg[c * n_per:(c + 1) * n_per]
        o = np.argsort(-d, kind="stable")
        orders.append(o)
        dsort[c, :n_per] = d[o]
    env = dsort.max(axis=0)
    chunks, W = plan_chunks(env, npp, nq, l_budget=l_budget)

    # per-column lookup tables for the slot layout
    col2off = np.zeros(npp, np.int64)
    col2g = np.zeros(npp, np.int64)
    col2cg = np.zeros(npp, np.int64)   # per-channel stride C*G
    col2cola = np.zeros(npp, np.int64)
    for (q, colq, c, g, off) in chunks:
        c0 = q * qc + colq
        for k in range(c):
            col2off[c0 + k] = off
            col2g[c0 + k] = g
            col2cg[c0 + k] = c * g
            col2cola[c0 + k] = k

    b1_eff = (b1 + u[0] * W1[HF]).astype(np.float32).reshape(H, 1)
    w1_16 = _to_bf16(np.ascontiguousarray(W1[:HF].astype(np.float32)))
    w2_16 = _to_bf16(np.ascontiguousarray(W2.astype(np.float32)))
    b2_c = np.ascontiguousarray(b2.astype(np.float32).reshape(F_X, 1))

    bounds = np.searchsorted(sc, np.arange(0, n_nodes + 1, n_per))
    in_maps = []
    for c in range(n_cores):
        o = orders[c]
        rank = np.empty(n_per, np.int64)
        rank[o] = np.arange(n_per)
        e0, e1 = bounds[c], bounds[c + 1]
        r = rank[sc[e0:e1].astype(np.int64) - c * n_per]
        p = r & 127
        colg = r >> 7
        pos0 = (col2off[colg] + col2cola[colg] * col2g[colg]
                + within[e0:e1])
        cg = col2cg[colg]
        stream = np.zeros((128, W), np.uint8)
        flat = (p * W + pos0)[:, None] + cg[:, None] * np.arange(NCH)
        stream.ravel()[flat] = msg8[e0:e1]

        xs = np.zeros((npad, F_X), np.float32)
        xs[:n_per] = x[c * n_per:(c + 1) * n_per][o]
        cnts = np.zeros(npad, np.float32)
        cnts[:n_per] = deg[c * n_per:(c + 1) * n_per][o]
        # rank r -> partition r%128, column r//128; [128, nq, F_X, qc]
        xq_arr = xs.reshape(nq, qc, 128, F_X).transpose(2, 0, 3, 1)
        cq_arr = cnts.reshape(nq, qc, 128).transpose(2, 0, 1)
        in_maps.append({
            "streamP": stream,
            "xq": np.ascontiguousarray(xq_arr),
            "cntq": np.ascontiguousarray(cq_arr),
            "w1": w1_16, "b1": b1_eff, "w2": w2_16, "b2": b2_c,
        })
    meta = dict(chunks=chunks, W=W, orders=orders, npp=npp, nq=nq)
    return in_maps, meta


def assemble_output(results, meta, n_nodes=N_NODES, n_cores=N_CORES):
    n_per = n_nodes // n_cores
    parts = []
    for c in range(n_cores):
        o = results[c]["outP"]  # [F_X, npad]
        res = np.empty((n_per, F_X), np.float32)
        res[meta["orders"][c]] = o[:, :n_per].T
        parts.append(res)
    return np.concatenate(parts, 0)


LAST_RUN = {}


def kernel(x, edge_index, edge_attr, u, batch, W1, b1, W2, b2):
    x = np.asarray(x, np.float32)
    edge_attr = np.asarray(edge_attr, np.float32)
    u = np.asarray(u, np.float32)
    W1 = np.asarray(W1, np.float32)
    b1 = np.asarray(b1, np.float32)
    W2 = np.asarray(W2, np.float32)
    b2 = np.asarray(b2, np.float32)
    row = np.asarray(edge_index[0]).astype(np.int64)
    col = np.asarray(edge_index[1]).astype(np.int64)

    in_maps, meta = prep_core_inputs(x, row, col, edge_attr, W1, b1, W2, b2, u)
    nc = build_kernel(meta["npp"], meta["nq"], meta["chunks"], meta["W"])
    import ml_dtypes
    for m in in_maps:
        m["streamP"] = m["streamP"].view(ml_dtypes.float8_e4m3)
        m["w1"] = m["w1"].view(ml_dtypes.bfloat16)
        m["w2"] = m["w2"].view(ml_dtypes.bfloat16)
    res = run_bass_kernel_spmd(nc, in_maps, core_ids=list(range(N_CORES)))
    LAST_RUN.update(nc=nc, in_maps=in_maps, meta=meta)
    return assemble_output(res.results, meta).astype(np.float32)


# revision 33
# speedup vs baseline: 1.2543x; 1.2543x over previous
"""Trainium2 Bass kernel for nn_NodeModel (GNN scatter-mean + node MLP).

Self-contained: takes FULL inputs as numpy arrays, shards by destination-node
range across 8 NeuronCores, runs a Bass/Tile kernel per core via
run_bass_kernel_spmd, and reassembles the full [500000, 8] output.

Strategy: nodes sharded by destination range (62500/core, no collectives).
The host sorts edges by destination, degree-sorts nodes within each core, and
packs the per-edge message [x[row] | edge_attr] (16 ch, bf16) into per-chunk
slot arrays whose slot count G tracks the local max degree (~33 avg instead of
the global max ~70), laid out partition-major so every stream DMA is
[128 partitions x large-contiguous].  Per-node counts (already computed for
the layout) ship as a tiny side input.

Device per core: chunked DMA -> one reduce_sum per chunk (DVE) over the slot
axis -> mean via max/reciprocal/multiply -> PE transposes of 128-node feature
columns -> PE matmuls for the 2-layer MLP (W1 24x25, W2 25x8, bf16), ACT for
bias+ReLU and PSUM evacuation.  Output is [8, npad] channel-major; the host
transposes and un-permutes the degree sort.
"""
from contextlib import ExitStack

import numpy as np

import concourse.bacc as bacc
import concourse.mybir as mybir
import concourse.tile as tile
from concourse.bass_utils import run_bass_kernel_spmd
from concourse.masks import make_identity

F_X = 8
F_E = 8
NCH = F_X + F_E          # 16 summed message channels
HF = F_X + NCH           # 24 feature channels into the MLP
H = 25
N_CORES = 8
N_NODES = 500_000
N_PER = N_NODES // N_CORES   # 62500
NQ = 4                       # quarters (pipeline granularity)
NPP = 492                    # node columns per core (492*128 = 62976 >= 62500)
L_BUDGET = 8448              # bf16 elems per partition per stream chunk


def plan_chunks(env, npp, nq, l_budget=L_BUDGET):
    """env: [npp*128] descending max-degree envelope (shared across cores).
    Returns ([(q, col_in_q, C, G, off)], total_W). One chunk = C node columns
    sharing slot count G; per-partition layout [ch][col][slot]."""
    qc = npp // nq
    chunks = []
    off = 0
    for q in range(nq):
        col = 0
        while col < qc:
            g = max(1, int(env[(q * qc + col) * 128]))
            c = max(1, min(qc - col, l_budget // (NCH * g)))
            chunks.append((q, col, c, g, off))
            off += NCH * c * g
            col += c
    return chunks, off


def build_kernel(npp, nq, chunks, W, repeat=1, do_reduce=True, do_mlp=True,
                 st_bufs=3):
    qc = npp // nq
    dt = mybir.dt
    nc = bacc.Bacc("TRN2", target_bir_lowering=False)

    streamP = nc.dram_tensor("streamP", [128, W], dt.bfloat16,
                             kind="ExternalInput")
    xq = nc.dram_tensor("xq", [128, nq, F_X, qc], dt.float32,
                        kind="ExternalInput")
    cntq = nc.dram_tensor("cntq", [128, nq, qc], dt.float32,
                          kind="ExternalInput")
    w1 = nc.dram_tensor("w1", [HF, H], dt.bfloat16, kind="ExternalInput")
    b1 = nc.dram_tensor("b1", [H, 1], dt.float32, kind="ExternalInput")
    w2 = nc.dram_tensor("w2", [H, F_X], dt.bfloat16, kind="ExternalInput")
    b2 = nc.dram_tensor("b2", [F_X, 1], dt.float32, kind="ExternalInput")
    outP = nc.dram_tensor("outP", [F_X, npp * 128], dt.float32,
                          kind="ExternalOutput")

    st_size = max(L_BUDGET, max(NCH * c * g for (_, _, c, g, _) in chunks))
    relu = mybir.ActivationFunctionType.Relu
    identf = mybir.ActivationFunctionType.Identity

    with tile.TileContext(nc) as tc, ExitStack() as ctx:
        const = ctx.enter_context(tc.tile_pool(name="const", bufs=1))
        persist = ctx.enter_context(tc.tile_pool(name="persist", bufs=1))
        sp = ctx.enter_context(tc.tile_pool(name="stream", bufs=st_bufs))
        msb = ctx.enter_context(tc.tile_pool(name="mlp", bufs=2))
        obp = ctx.enter_context(tc.tile_pool(name="outb", bufs=2))
        psum = ctx.enter_context(tc.tile_pool(name="psum", bufs=2,
                                              space="PSUM"))

        ident = const.tile([128, 128], dt.float32)
        make_identity(nc, ident)
        w1t = const.tile([HF, H], dt.bfloat16)
        nc.sync.dma_start(out=w1t[:], in_=w1[:])
        b1t = const.tile([H, 1], dt.float32)
        nc.sync.dma_start(out=b1t[:], in_=b1[:])
        w2t = const.tile([H, F_X], dt.bfloat16)
        nc.sync.dma_start(out=w2t[:], in_=w2[:])
        b2t = const.tile([F_X, 1], dt.float32)
        nc.sync.dma_start(out=b2t[:], in_=b2[:])

        by_q = {q: [ch for ch in chunks if ch[0] == q] for q in range(nq)}

        for q in [q for _ in range(repeat) for q in range(nq)]:
            feat = persist.tile([128, HF, qc], dt.float32, tag=f"feat{q}")
            accum = persist.tile([128, NCH, qc], dt.float32, tag=f"acc{q}")
            inv = persist.tile([128, qc], dt.float32, tag=f"inv{q}")

            # scalar (ACT) HWDGE ring: keeps these off the SP ring so a
            # queued wait can't stall the stream DMAs behind it
            nc.scalar.dma_start(out=feat[:, 0:F_X, :], in_=xq[:, q])
            nc.scalar.dma_start(out=inv[:], in_=cntq[:, q])
            nc.vector.tensor_scalar_max(out=inv[:], in0=inv[:], scalar1=1.0)
            nc.vector.reciprocal(out=inv[:], in_=inv[:])

            if do_reduce:
                for (_, col, c, g, off) in by_q[q]:
                    stt = sp.tile([128, st_size], dt.bfloat16, tag="st")
                    n = NCH * c * g
                    nc.sync.dma_start(out=stt[:, :n],
                                      in_=streamP[:, off:off + n])
                    nc.vector.reduce_sum(
                        out=accum[:, :, col:col + c],
                        in_=stt[:, :n].rearrange("p (f c g) -> p f c g",
                                                 f=NCH, c=c),
                        axis=mybir.AxisListType.X,
                    )

                for ci in range(NCH):
                    nc.vector.tensor_tensor(
                        out=feat[:, F_X + ci, :], in0=accum[:, ci, :],
                        in1=inv[:], op=mybir.AluOpType.mult,
                    )

            if not do_mlp:  # timing probe only: skip MLP, output stays zero
                continue

            # ---- MLP over this quarter, blocks of up to 4 node columns ----
            ob = None
            ob_base = 0
            for b0 in range(0, qc, 4):
                bc = min(4, qc - b0)
                n = bc * 128
                if (b0 // 4) % 4 == 0:
                    ob = obp.tile([F_X, 2048], dt.float32, tag="ob")
                    ob_base = b0
                ftp = psum.tile([HF, 512], dt.float32, tag="ft")
                for i in range(bc):
                    nc.tensor.transpose(ftp[:, i * 128:(i + 1) * 128],
                                        feat[:, :, b0 + i], ident)
                fts = msb.tile([HF, 512], dt.bfloat16, tag="fts")
                nc.scalar.copy(out=fts[:, :n], in_=ftp[:, :n])
                hp = psum.tile([H, 512], dt.float32, tag="h")
                nc.tensor.matmul(hp[:, :n], w1t[:], fts[:, :n],
                                 start=True, stop=True)
                hs = msb.tile([H, 512], dt.bfloat16, tag="hs")
                nc.scalar.activation(hs[:, :n], hp[:, :n], relu, bias=b1t[:])
                op_ = psum.tile([F_X, 512], dt.float32, tag="o")
                nc.tensor.matmul(op_[:, :n], w2t[:], hs[:, :n],
                                 start=True, stop=True)
                oc = (b0 - ob_base) * 128
                nc.scalar.activation(ob[:, oc:oc + n], op_[:, :n], identf,
                                     bias=b2t[:])
                if (b0 // 4) % 4 == 3 or b0 + bc >= qc:
                    done = (b0 + bc - ob_base) * 128
                    base = (q * qc + ob_base) * 128
                    nc.scalar.dma_start(out=outP[:, base:base + done],
                                        in_=ob[:, :done])

    nc.compile()
    return nc


def _to_bf16(a_f32):
    """f32 -> bf16 (round-to-nearest-even) as uint16 view."""
    u = np.ascontiguousarray(a_f32).view(np.uint32)
    rounded = (u + 0x7FFF + ((u >> 16) & 1)) >> 16
    return rounded.astype(np.uint16)


def prep_stage1(x, row, col, edge_attr, n_nodes=N_NODES):
    """Layout-independent prep: destination sort + bf16 message table."""
    deg = np.bincount(col, minlength=n_nodes).astype(np.int64)
    order = np.argsort(col.astype(np.int32), kind="stable")
    sc = col.astype(np.int32)[order]
    starts = np.zeros(n_nodes + 1, np.int64)
    starts[1:] = np.cumsum(deg)
    within = np.arange(len(col), dtype=np.int64) - starts[sc]
    x16 = _to_bf16(x.astype(np.float32))
    ea16 = _to_bf16(edge_attr.astype(np.float32))
    msg16 = np.empty((len(col), NCH), np.uint16)
    msg16[:, :F_X] = x16[row[order]]
    msg16[:, F_X:] = ea16[order]
    return dict(deg=deg, sc=sc, within=within, msg16=msg16)


def prep_core_inputs(x, row, col, edge_attr, W1, b1, W2, b2, u,
                     n_nodes=N_NODES, n_cores=N_CORES, npp=NPP, nq=NQ,
                     l_budget=L_BUDGET, stage1=None):
    n_per = n_nodes // n_cores
    npad = npp * 128
    qc = npp // nq
    if stage1 is None:
        stage1 = prep_stage1(x, row, col, edge_attr, n_nodes=n_nodes)
    deg = stage1["deg"]
    sc = stage1["sc"]
    within = stage1["within"]
    msg16 = stage1["msg16"]

    # per-core degree sort; shared descending max-degree envelope
    orders = []
    dsort = np.zeros((n_cores, npad), np.int64)
    for c in range(n_cores):
        d = deg[c * n_per:(c + 1) * n_per]
        o = np.argsort(-d, kind="stable")
        orders.append(o)
        dsort[c, :n_per] = d[o]
    env = dsort.max(axis=0)
    chunks, W = plan_chunks(env, npp, nq, l_budget=l_budget)

    # per-column lookup tables for the slot layout
    col2off = np.zeros(npp, np.int64)
    col2g = np.zeros(npp, np.int64)
    col2cg = np.zeros(npp, np.int64)   # per-channel stride C*G
    col2cola = np.zeros(npp, np.int64)
    for (q, colq, c, g, off) in chunks:
        c0 = q * qc + colq
        for k in range(c):
            col2off[c0 + k] = off
            col2g[c0 + k] = g
            col2cg[c0 + k] = c * g
            col2cola[c0 + k] = k

    b1_eff = (b1 + u[0] * W1[HF]).astype(np.float32).reshape(H, 1)
    w1_16 = _to_bf16(np.ascontiguousarray(W1[:HF].astype(np.float32)))
    w2_16 = _to_bf16(np.ascontiguousarray(W2.astype(np.float32)))
    b2_c = np.ascontiguousarray(b2.astype(np.float32).reshape(F_X, 1))

    bounds = np.searchsorted(sc, np.arange(0, n_nodes + 1, n_per))
    in_maps = []
    for c in range(n_cores):
        o = orders[c]
        rank = np.empty(n_per, np.int64)
        rank[o] = np.arange(n_per)
        e0, e1 = bounds[c], bounds[c + 1]
        r = rank[sc[e0:e1].astype(np.int64) - c * n_per]
        p = r & 127
        colg = r >> 7
        pos0 = (col2off[colg] + col2cola[colg] * col2g[colg]
                + within[e0:e1])
        cg = col2cg[colg]
        stream = np.zeros((128, W), np.uint16)
        flat = (p * W + pos0)[:, None] + cg[:, None] * np.arange(NCH)
        stream.ravel()[flat] = msg16[e0:e1]

        xs = np.zeros((npad, F_X), np.float32)
        xs[:n_per] = x[c * n_per:(c + 1) * n_per][o]
        cnts = np.zeros(npad, np.float32)
        cnts[:n_per] = deg[c * n_per:(c + 1) * n_per][o]
        # rank r -> partition r%128, column r//128; [128, nq, F_X, qc]
        xq_arr = xs.reshape(nq, qc, 128, F_X).transpose(2, 0, 3, 1)
        cq_arr = cnts.reshape(nq, qc, 128).transpose(2, 0, 1)
        in_maps.append({
            "streamP": stream,
            "xq": np.ascontiguousarray(xq_arr),
            "cntq": np.ascontiguousarray(cq_arr),
            "w1": w1_16, "b1": b1_eff, "w2": w2_16, "b2": b2_c,
        })
    meta = dict(chunks=chunks, W=W, orders=orders, npp=npp, nq=nq)
    return in_maps, meta


def assemble_output(results, meta, n_nodes=N_NODES, n_cores=N_CORES):
    n_per = n_nodes // n_cores
    parts = []
    for c in range(n_cores):
        o = results[c]["outP"]  # [F_X, npad]
        res = np.empty((n_per, F_X), np.float32)
        res[meta["orders"][c]] = o[:, :n_per].T
        parts.append(res)
    return np.concatenate(parts, 0)


LAST_RUN = {}


def kernel(x, edge_index, edge_attr, u, batch, W1, b1, W2, b2):
    x = np.asarray(x, np.float32)
    edge_attr = np.asarray(edge_attr, np.float32)
    u = np.asarray(u, np.float32)
    W1 = np.asarray(W1, np.float32)
    b1 = np.asarray(b1, np.float32)
    W2 = np.asarray(W2, np.float32)
    b2 = np.asarray(b2, np.float32)
    row = np.asarray(edge_index[0]).astype(np.int64)
    col = np.asarray(edge_index[1]).astype(np.int64)

    in_maps, meta = prep_core_inputs(x, row, col, edge_attr, W1, b1, W2, b2, u)
    nc = build_kernel(meta["npp"], meta["nq"], meta["chunks"], meta["W"])
    import ml_dtypes
    for m in in_maps:
        m["streamP"] = m["streamP"].view(ml_dtypes.bfloat16)
        m["w1"] = m["w1"].view(ml_dtypes.bfloat16)
        m["w2"] = m["w2"].view(ml_dtypes.bfloat16)
    res = run_bass_kernel_spmd(nc, in_maps, core_ids=list(range(N_CORES)))
    LAST_RUN.update(nc=nc, in_maps=in_maps, meta=meta)
    return assemble_output(res.results, meta).astype(np.float32)
